# revision 25
# baseline (speedup 1.0000x reference)
import sys

for _p in ("/root/.axon_site/_ro/trn_rl_repo", "/opt/trn_rl_repo"):
    if _p not in sys.path:
        sys.path.insert(0, _p)

import numpy as np
import ml_dtypes

# Problem constants (nn_LocalConvolution): x [4,256,64,64] f32,
# weight [4,1,16,49,64,64] f32, K=7, pad=3, stride=1, dil=1.
# out[b, g*16+cc, y, x] = sum_k x_pad[b, g*16+cc, y+kh-3, x+kw-3] * w[b,0,cc,k,y,x]
B, C, H, W = 4, 256, 64, 64
WC, KK, K, PAD = 16, 49, 7, 3
G = C // WC  # 16 channel groups sharing each weight channel
NCORES = 8
HHALF = H // 2  # 32 output rows per core (B=4 x 2 H-halves = 8 shards)
PART = 128
# Partition p = cc*8 + yb: weight channel cc in [0,16), y-block yb in [0,8).
# Each partition computes 4 output rows (yq) x 16 groups (g) x 64 cols.
NYB = 8  # y-blocks per core
YQ = 4  # rows per y-block
ROWS = YQ + K - 1  # 10 halo rows of padded input per partition
XC = W + 2 * PAD  # 70 padded cols
FREE = YQ * G * W  # 4096 product elems per partition per tap
BANK = 512  # fp32 elems per PSUM bank
NBANK = FREE // BANK  # 8 banks

# Tap processing order: even kw first (xb only needed for odd kw, so its
# DMA is off the critical path), kh-major so weight chunk kh=0 unblocks
# the first taps.
TAP_SEQ = [(kh, kw) for kh in range(K) for kw in (0, 2, 4, 6)] + [
    (kh, kw) for kh in range(K) for kw in (1, 3, 5)
]
# Positions in TAP_SEQ executed on GPSIMD instead of DVE. Measured: GPSIMD
# tensor_tensor contends for the shared SBUF port and slows concurrent DVE
# ops ~4x — keep this empty.
GP_POS = frozenset()
# xa DMA row chunks: tap 0's first half (yq 0-1) only needs rows 0-1
XCHUNKS = ((0, 2), (2, 4), (4, 7), (7, 10))
# weight DMA chunks over TAP_SEQ positions (tap-order layout in DRAM);
# tiny first chunk so tap 0 starts as early as possible
WCHUNKS = ((0, 1), (1, 4), (4, 10), (10, 16), (16, 22), (22, 28), (28, 36), (36, 49))

_BF16 = ml_dtypes.bfloat16
_cache = {}


def _build():
    import concourse.bacc as bacc
    import concourse.mybir as mybir
    import concourse.tile as tile

    nc = bacc.Bacc(None, target_bir_lowering=False)
    bf = mybir.dt.bfloat16
    f32 = mybir.dt.float32

    xa_d = nc.dram_tensor("xa", (PART, ROWS * G * XC), bf, kind="ExternalInput")
    xb_d = nc.dram_tensor("xb", (PART, ROWS * G * XC), bf, kind="ExternalInput")
    wr_d = nc.dram_tensor("wr", (PART, KK * YQ * W), bf, kind="ExternalInput")
    id_d = nc.dram_tensor("ident", (PART, PART), bf, kind="ExternalInput")
    out_d = nc.dram_tensor("out", (PART, FREE), bf, kind="ExternalOutput")

    with tile.TileContext(nc) as tc:
        with (
            tc.tile_pool(name="xpool", bufs=1) as xpool,
            tc.tile_pool(name="cpool", bufs=1) as cpool,
            tc.tile_pool(name="wpool", bufs=1) as wpool,
            tc.tile_pool(name="tpool", bufs=6) as tpool,
            tc.tile_pool(name="gpool", bufs=2) as gpool,
            tc.tile_pool(name="opool", bufs=1) as opool,
            tc.tile_pool(name="psum", bufs=1, space="PSUM") as ppool,
        ):
            xa_t = xpool.tile([PART, ROWS, G, XC], bf, tag="xa")
            xb_t = xpool.tile([PART, ROWS, G, XC], bf, tag="xb")
            id_t = cpool.tile([PART, PART], bf, tag="id")
            warm = cpool.tile([PART, 1], f32, tag="warm")
            # critical path: ident then xa on the sync queue; weights on the
            # scalar queue in parallel; xb (odd-kw phase) last.
            # All input DMAs on one ring (sync): FIFO order = bandwidth
            # priority, and a single ring gets the full HBM rate. Order:
            # ident, xa rows 0-3, first weight taps, then interleaved.
            w_tiles = {}
            w_of_pos = {}
            for lo, hi in WCHUNKS:
                w_t = wpool.tile([PART, hi - lo, YQ, 1, W], bf, tag=f"w{lo}")
                w_tiles[lo] = w_t
                for pos in range(lo, hi):
                    w_of_pos[pos] = (w_t, pos - lo)

            def w_dma(ci):
                lo, hi = WCHUNKS[ci]
                nc.sync.dma_start(
                    w_tiles[lo][:], wr_d[:, lo * YQ * W : hi * YQ * W]
                )

            def xa_dma(ci):
                r0, r1 = XCHUNKS[ci]
                nc.sync.dma_start(
                    xa_t[:, r0:r1, :, :], xa_d[:, r0 * G * XC : r1 * G * XC]
                )

            nc.sync.dma_start(id_t[:], id_d[:])
            w_dma(0)
            xa_dma(0)
            xa_dma(1)
            w_dma(1)
            xa_dma(2)
            w_dma(2)
            xa_dma(3)
            w_dma(3)
            nc.sync.dma_start(xb_t[:], xb_d[:])
            for ci in range(4, len(WCHUNKS)):
                w_dma(ci)

            # preload the ACT copy table set during the head so the tail's
            # PSUM->SBUF copies don't pay ACT_TABLE_LOAD
            nc.scalar.copy(warm[:], id_t[:, 0:1])

            acc = [
                ppool.tile([PART, BANK], f32, name=f"ps{j}", tag=f"ps{j}")
                for j in range(NBANK)
            ]

            def tap_src(pos):
                kh, kw = TAP_SEQ[pos]
                if kw % 2 == 0:
                    src = xa_t[:, kh : kh + YQ, :, kw : kw + W]
                else:
                    src = xb_t[:, kh : kh + YQ, :, kw + 1 : kw + 1 + W]
                w_t, i = w_of_pos[pos]
                wap = w_t[:, i, :, :, :].broadcast_to((PART, YQ, G, W))
                return src, wap

            gp_pos = sorted(GP_POS)
            gp_tmp = {}

            def gp_issue(pos):
                src, wap = tap_src(pos)
                tmp = gpool.tile([PART, YQ, G, W], bf, tag="gtmp")
                nc.gpsimd.tensor_mul(tmp[:], src, wap)
                gp_tmp[pos] = tmp

            if gp_pos:
                gp_issue(gp_pos[0])

            def emit_tap_split(pos):
                # two yq-halves: the first tap starts on 2 input rows only,
                # the last tap lets banks 0-3 finish/copy while half B runs
                kh, kw = TAP_SEQ[pos]
                w_t, wi = w_of_pos[pos]
                for half in range(2):
                    q0 = half * 2
                    if kw % 2 == 0:
                        src = xa_t[:, kh + q0 : kh + q0 + 2, :, kw : kw + W]
                    else:
                        src = xb_t[:, kh + q0 : kh + q0 + 2, :, kw + 1 : kw + 1 + W]
                    wap = w_t[:, wi, q0 : q0 + 2, :, :].broadcast_to(
                        (PART, 2, G, W)
                    )
                    tmp = tpool.tile([PART, 2, G, W], bf, tag="tmp2")
                    nc.vector.tensor_mul(tmp[:], src, wap)
                    for jj in range(4):
                        j = half * 4 + jj
                        yq, gh = jj // 2, jj % 2
                        nc.tensor.matmul(
                            acc[j][:],
                            id_t[:],
                            tmp[:, yq, gh * 8 : (gh + 1) * 8, :],
                            start=(pos == 0),
                            stop=(pos == KK - 1),
                        )

            emit_tap_split(0)
            for pos in range(1, KK - 1):
                if pos in GP_POS:
                    # product was issued one gp-slot ago; prefetch the next
                    i = gp_pos.index(pos)
                    if i + 1 < len(gp_pos):
                        gp_issue(gp_pos[i + 1])
                    tmp = gp_tmp.pop(pos)
                else:
                    src, wap = tap_src(pos)
                    tmp = tpool.tile([PART, YQ, G, W], bf, tag="tmp")
                    nc.vector.tensor_mul(tmp[:], src, wap)
                for j in range(NBANK):
                    yq, gh = j // 2, j % 2
                    nc.tensor.matmul(
                        acc[j][:],
                        id_t[:],
                        tmp[:, yq, gh * 8 : (gh + 1) * 8, :],
                        start=False,
                        stop=False,
                    )
            emit_tap_split(KK - 1)

            ostg = opool.tile([PART, NBANK, BANK], bf, tag="ostg")
            for j in range(NBANK):
                if j % 2 == 0:
                    nc.scalar.copy(ostg[:, j, :], acc[j][:])
                else:
                    nc.vector.tensor_copy(ostg[:, j, :], acc[j][:])
            for q in range(4):
                lo, hi = q * 2, q * 2 + 2
                nc.sync.dma_start(
                    out_d[:, lo * BANK : hi * BANK], ostg[:, lo:hi, :]
                )

    _dedupe_ldweights(nc)
    _hoist_input_dmas(nc)
    nc.compile()
    return nc


def _hoist_input_dmas(nc):
    """Move the leading wait-free input DMA issues from the body block to
    before the SP engine's all-engine-barrier wait in the preamble block, so
    the transfers overlap the ~7us engine-start preamble."""
    blocks = nc.main_func.blocks
    if len(blocks) < 2:
        return 0
    b0, b1 = blocks[0], blocks[1]
    hoist, keep = [], []
    frontier = True
    for ins in b1.instructions:
        if (
            frontier
            and type(ins).__name__ == "InstDMACopy"
            and getattr(ins.engine, "value", None) == "SP"
            and not (ins.sync_info and ins.sync_info.on_wait)
        ):
            hoist.append(ins)
            continue
        if type(ins).__name__ == "InstDMACopy" and getattr(
            ins.engine, "value", None
        ) == "SP":
            # stop at the first waiting DMA: later ones may reuse sem lanes
            frontier = False
        keep.append(ins)
    if not hoist:
        return 0
    # insertion point: first SP instruction in b0 that waits (barrier entry)
    idx = None
    for i, ins in enumerate(b0.instructions):
        if (
            getattr(ins.engine, "value", None) == "SP"
            and ins.sync_info
            and ins.sync_info.on_wait
        ):
            idx = i
            break
    if idx is None:
        idx = len(b0.instructions)
    b0.instructions[idx:idx] = hoist
    b1.instructions[:] = keep
    return len(hoist)


def _dedupe_ldweights(nc):
    """All PE matmuls share one identity stationary; drop every InstLdweights
    after the first so the PE array keeps the loaded weights. Only removes
    LdWeights that carry no semaphore activity and whose AP matches the
    first one exactly."""
    first_repr = None
    removed = 0
    for blk in nc.main_func.blocks:
        keep = []
        for inst in blk.instructions:
            if type(inst).__name__ == "InstLdweights":
                si = inst.sync_info
                clean = si is None or (not si.on_wait and not si.on_update)
                r = repr(inst.ins[0])
                if first_repr is None:
                    first_repr = r
                elif clean and r == first_repr:
                    removed += 1
                    continue
            keep.append(inst)
        blk.instructions[:] = keep
    return removed


def _prep_core(x, w, b, h):
    """Host-side shard prep for one core: pad/slab x, reshape w.

    Layout: partition p = cc*8 + yb. xa[p] = padded rows
    [h*32+yb*4, +10) of channels {g*16+cc}, shape [ROWS, G, XC].
    xb = xa shifted right by one column (for odd-kw 4B alignment).
    wr[p] = weight[b,0,cc,:,h*32+yb*4:+4,:], shape [KK, YQ, W].
    """
    xpad = np.zeros((C, H + 2 * PAD, XC), dtype=np.float32)
    xpad[:, PAD : PAD + H, PAD : PAD + W] = x[b]
    v = xpad[:, h * HHALF : h * HHALF + HHALF + 2 * PAD, :]  # [C, 38, 70]
    vr = v.reshape(G, WC, HHALF + 2 * PAD, XC)  # [g, cc, 38, 70]
    xa = np.empty((WC, NYB, ROWS, G, XC), dtype=np.float32)
    for yb in range(NYB):
        # [g, cc, 10, 70] -> [cc, 10, g, 70]
        xa[:, yb] = vr[:, :, yb * YQ : yb * YQ + ROWS, :].transpose(1, 2, 0, 3)
    xa = xa.reshape(PART, -1)
    xb = np.zeros_like(xa)
    xb3 = xb.reshape(PART, ROWS * G, XC)
    xb3[:, :, 1:] = xa.reshape(PART, ROWS * G, XC)[:, :, :-1]
    # weights: [16, 49, 32, 64] -> [cc, yb, pos, yq, x] -> [128, 49*4*64]
    # with the tap axis permuted into TAP_SEQ order
    wsh = w[b, 0, :, :, h * HHALF : (h + 1) * HHALF, :]
    wr = wsh.reshape(WC, KK, NYB, YQ, W).transpose(0, 2, 1, 3, 4)
    perm = [kh * K + kw for kh, kw in TAP_SEQ]
    wr = wr[:, :, perm, :, :]
    return (
        xa.astype(_BF16),
        xb.astype(_BF16),
        np.ascontiguousarray(wr).reshape(PART, -1).astype(_BF16),
    )


def _prep_inputs(x, weight):
    ident = np.eye(PART, dtype=_BF16)
    in_maps = []
    for core in range(NCORES):
        b, h = core // 2, core % 2
        xa, xb, wr = _prep_core(x, weight, b, h)
        in_maps.append({"xa": xa, "xb": xb, "wr": wr, "ident": ident})
    return in_maps


def _unpack_out(results):
    out = np.empty((B, C, H, W), dtype=np.float32)
    for core in range(NCORES):
        b, h = core // 2, core % 2
        # [cc, yb, yq, gh, g8, x] -> c = gh*128 + g8*16 + cc, y = yb*4+yq
        o = results[core]["out"].astype(np.float32).reshape(WC, NYB, YQ, 2, 8, W)
        o = o.transpose(3, 4, 0, 1, 2, 5).reshape(C, HHALF, W)
        out[b, :, h * HHALF : (h + 1) * HHALF, :] = o
    return out


def kernel(x: np.ndarray, weight: np.ndarray) -> np.ndarray:
    from concourse.bass_utils import run_bass_kernel_spmd

    if "nc" not in _cache:
        _cache["nc"] = _build()
    nc = _cache["nc"]

    in_maps = _prep_inputs(x, weight)
    res = run_bass_kernel_spmd(nc, in_maps, list(range(NCORES)))
    return _unpack_out(res.results)


# revision 29
# speedup vs baseline: 1.0081x; 1.0081x over previous
import sys

for _p in ("/root/.axon_site/_ro/trn_rl_repo", "/opt/trn_rl_repo"):
    if _p not in sys.path:
        sys.path.insert(0, _p)

import numpy as np
import ml_dtypes

# Problem constants (nn_LocalConvolution): x [4,256,64,64] f32,
# weight [4,1,16,49,64,64] f32, K=7, pad=3, stride=1, dil=1.
# out[b, g*16+cc, y, x] = sum_k x_pad[b, g*16+cc, y+kh-3, x+kw-3] * w[b,0,cc,k,y,x]
B, C, H, W = 4, 256, 64, 64
WC, KK, K, PAD = 16, 49, 7, 3
G = C // WC  # 16 channel groups sharing each weight channel
NCORES = 8
HHALF = H // 2  # 32 output rows per core (B=4 x 2 H-halves = 8 shards)
PART = 128
# Partition p = cc*8 + yb: weight channel cc in [0,16), y-block yb in [0,8).
# Each partition computes 4 output rows (yq) x 16 groups (g) x 64 cols.
NYB = 8  # y-blocks per core
YQ = 4  # rows per y-block
ROWS = YQ + K - 1  # 10 halo rows of padded input per partition
XC = W + 2 * PAD  # 70 padded cols
FREE = YQ * G * W  # 4096 product elems per partition per tap
BANK = 512  # fp32 elems per PSUM bank
NBANK = FREE // BANK  # 8 banks

# Tap processing order: even kw first (xb only needed for odd kw, so its
# DMA is off the critical path), kh-major so weight chunk kh=0 unblocks
# the first taps.
TAP_SEQ = [(kh, kw) for kh in range(K) for kw in (0, 2, 4, 6)] + [
    (kh, kw) for kh in range(K) for kw in (1, 3, 5)
]
# Positions in TAP_SEQ executed on GPSIMD instead of DVE. Measured: GPSIMD
# tensor_tensor contends for the shared SBUF port and slows concurrent DVE
# ops ~4x — keep this empty.
GP_POS = frozenset()
# xa DMA row chunks: first taps (kh=0) only need rows 0-3
XCHUNKS = ((0, 4), (4, 7), (7, 10))
# weight DMA chunks over TAP_SEQ positions (tap-order layout in DRAM);
# tiny first chunk so tap 0 starts as early as possible
WCHUNKS = ((0, 1), (1, 4), (4, 10), (10, 16), (16, 22), (22, 28), (28, 36), (36, 49))

_BF16 = ml_dtypes.bfloat16
_cache = {}


def _build():
    import concourse.bacc as bacc
    import concourse.mybir as mybir
    import concourse.tile as tile

    nc = bacc.Bacc(None, target_bir_lowering=False)
    bf = mybir.dt.bfloat16
    f32 = mybir.dt.float32

    xa_d = nc.dram_tensor("xa", (PART, ROWS * G * XC), bf, kind="ExternalInput")
    xb_d = nc.dram_tensor("xb", (PART, ROWS * G * XC), bf, kind="ExternalInput")
    wr_d = nc.dram_tensor("wr", (PART, KK * YQ * W), bf, kind="ExternalInput")
    id_d = nc.dram_tensor("ident", (PART, PART), bf, kind="ExternalInput")
    out_d = nc.dram_tensor("out", (PART, FREE), bf, kind="ExternalOutput")

    with tile.TileContext(nc) as tc:
        with (
            tc.tile_pool(name="xpool", bufs=1) as xpool,
            tc.tile_pool(name="cpool", bufs=1) as cpool,
            tc.tile_pool(name="wpool", bufs=1) as wpool,
            tc.tile_pool(name="tpool", bufs=6) as tpool,
            tc.tile_pool(name="gpool", bufs=2) as gpool,
            tc.tile_pool(name="opool", bufs=1) as opool,
            tc.tile_pool(name="psum", bufs=1, space="PSUM") as ppool,
        ):
            xa_t = xpool.tile([PART, ROWS, G, XC], bf, tag="xa")
            xb_t = xpool.tile([PART, ROWS, G, XC], bf, tag="xb")
            id_t = cpool.tile([PART, PART], bf, tag="id")
            warm = cpool.tile([PART, 1], f32, tag="warm")
            # critical path: ident then xa on the sync queue; weights on the
            # scalar queue in parallel; xb (odd-kw phase) last.
            # All input DMAs on one ring (sync): FIFO order = bandwidth
            # priority, and a single ring gets the full HBM rate. Order:
            # ident, xa rows 0-3, first weight taps, then interleaved.
            w_tiles = {}
            w_of_pos = {}
            for lo, hi in WCHUNKS:
                w_t = wpool.tile([PART, hi - lo, YQ, 1, W], bf, tag=f"w{lo}")
                w_tiles[lo] = w_t
                for pos in range(lo, hi):
                    w_of_pos[pos] = (w_t, pos - lo)

            def w_dma(ci):
                lo, hi = WCHUNKS[ci]
                nc.sync.dma_start(
                    w_tiles[lo][:], wr_d[:, lo * YQ * W : hi * YQ * W]
                )

            def xa_dma(ci):
                r0, r1 = XCHUNKS[ci]
                nc.sync.dma_start(
                    xa_t[:, r0:r1, :, :], xa_d[:, r0 * G * XC : r1 * G * XC]
                )

            nc.sync.dma_start(id_t[:], id_d[:])
            w_dma(0)
            xa_dma(0)
            w_dma(1)
            xa_dma(1)
            w_dma(2)
            xa_dma(2)
            w_dma(3)
            nc.sync.dma_start(xb_t[:], xb_d[:])
            for ci in range(4, len(WCHUNKS)):
                w_dma(ci)

            # preload the ACT copy table set during the head so the tail's
            # PSUM->SBUF copies don't pay ACT_TABLE_LOAD
            nc.scalar.copy(warm[:], id_t[:, 0:1])

            acc = [
                ppool.tile([PART, BANK], f32, name=f"ps{j}", tag=f"ps{j}")
                for j in range(NBANK)
            ]

            def tap_src(pos):
                kh, kw = TAP_SEQ[pos]
                if kw % 2 == 0:
                    src = xa_t[:, kh : kh + YQ, :, kw : kw + W]
                else:
                    src = xb_t[:, kh : kh + YQ, :, kw + 1 : kw + 1 + W]
                w_t, i = w_of_pos[pos]
                wap = w_t[:, i, :, :, :].broadcast_to((PART, YQ, G, W))
                return src, wap

            gp_pos = sorted(GP_POS)
            gp_tmp = {}

            def gp_issue(pos):
                src, wap = tap_src(pos)
                tmp = gpool.tile([PART, YQ, G, W], bf, tag="gtmp")
                nc.gpsimd.tensor_mul(tmp[:], src, wap)
                gp_tmp[pos] = tmp

            if gp_pos:
                gp_issue(gp_pos[0])

            def emit_tap_split(pos):
                # two yq-halves: the first tap starts on 2 input rows only,
                # the last tap lets banks 0-3 finish/copy while half B runs
                kh, kw = TAP_SEQ[pos]
                w_t, wi = w_of_pos[pos]
                for half in range(2):
                    q0 = half * 2
                    if kw % 2 == 0:
                        src = xa_t[:, kh + q0 : kh + q0 + 2, :, kw : kw + W]
                    else:
                        src = xb_t[:, kh + q0 : kh + q0 + 2, :, kw + 1 : kw + 1 + W]
                    wap = w_t[:, wi, q0 : q0 + 2, :, :].broadcast_to(
                        (PART, 2, G, W)
                    )
                    tmp = tpool.tile([PART, 2, G, W], bf, tag="tmp2")
                    nc.vector.tensor_mul(tmp[:], src, wap)
                    for jj in range(4):
                        j = half * 4 + jj
                        yq, gh = jj // 2, jj % 2
                        nc.tensor.matmul(
                            acc[j][:],
                            id_t[:],
                            tmp[:, yq, gh * 8 : (gh + 1) * 8, :],
                            start=(pos == 0),
                            stop=(pos == KK - 1),
                        )

            first_src, first_wap = tap_src(0)
            first_tmp = tpool.tile([PART, YQ, G, W], bf, tag="tmp")
            nc.vector.tensor_mul(first_tmp[:], first_src, first_wap)
            for j in range(NBANK):
                yq, gh = j // 2, j % 2
                nc.tensor.matmul(
                    acc[j][:],
                    id_t[:],
                    first_tmp[:, yq, gh * 8 : (gh + 1) * 8, :],
                    start=True,
                    stop=False,
                )
            for pos in range(1, KK - 1):
                if pos in GP_POS:
                    # product was issued one gp-slot ago; prefetch the next
                    i = gp_pos.index(pos)
                    if i + 1 < len(gp_pos):
                        gp_issue(gp_pos[i + 1])
                    tmp = gp_tmp.pop(pos)
                else:
                    src, wap = tap_src(pos)
                    tmp = tpool.tile([PART, YQ, G, W], bf, tag="tmp")
                    nc.vector.tensor_mul(tmp[:], src, wap)
                for j in range(NBANK):
                    yq, gh = j // 2, j % 2
                    nc.tensor.matmul(
                        acc[j][:],
                        id_t[:],
                        tmp[:, yq, gh * 8 : (gh + 1) * 8, :],
                        start=False,
                        stop=False,
                    )
            emit_tap_split(KK - 1)

            ostg = opool.tile([PART, NBANK, BANK], bf, tag="ostg")
            for j in range(NBANK):
                if j % 2 == 0:
                    nc.scalar.copy(ostg[:, j, :], acc[j][:])
                else:
                    nc.vector.tensor_copy(ostg[:, j, :], acc[j][:])
            for half in range(2):
                lo, hi = half * 4, half * 4 + 4
                nc.sync.dma_start(
                    out_d[:, lo * BANK : hi * BANK], ostg[:, lo:hi, :]
                )

    _dedupe_ldweights(nc)
    _hoist_input_dmas(nc)
    nc.compile()
    return nc


def _hoist_input_dmas(nc):
    """Move the leading wait-free input DMA issues from the body block to
    before the SP engine's all-engine-barrier wait in the preamble block, so
    the transfers overlap the ~7us engine-start preamble."""
    blocks = nc.main_func.blocks
    if len(blocks) < 2:
        return 0
    b0, b1 = blocks[0], blocks[1]
    hoist, keep = [], []
    frontier = True
    for ins in b1.instructions:
        if (
            frontier
            and type(ins).__name__ == "InstDMACopy"
            and getattr(ins.engine, "value", None) == "SP"
            and not (ins.sync_info and ins.sync_info.on_wait)
        ):
            hoist.append(ins)
            continue
        if type(ins).__name__ == "InstDMACopy" and getattr(
            ins.engine, "value", None
        ) == "SP":
            # stop at the first waiting DMA: later ones may reuse sem lanes
            frontier = False
        keep.append(ins)
    if not hoist:
        return 0
    # insertion point: first SP instruction in b0 that waits (barrier entry)
    idx = None
    for i, ins in enumerate(b0.instructions):
        if (
            getattr(ins.engine, "value", None) == "SP"
            and ins.sync_info
            and ins.sync_info.on_wait
        ):
            idx = i
            break
    if idx is None:
        idx = len(b0.instructions)
    b0.instructions[idx:idx] = hoist
    b1.instructions[:] = keep
    return len(hoist)


def _dedupe_ldweights(nc):
    """All PE matmuls share one identity stationary; drop every InstLdweights
    after the first so the PE array keeps the loaded weights. Only removes
    LdWeights that carry no semaphore activity and whose AP matches the
    first one exactly."""
    first_repr = None
    removed = 0
    for blk in nc.main_func.blocks:
        keep = []
        for inst in blk.instructions:
            if type(inst).__name__ == "InstLdweights":
                si = inst.sync_info
                clean = si is None or (not si.on_wait and not si.on_update)
                r = repr(inst.ins[0])
                if first_repr is None:
                    first_repr = r
                elif clean and r == first_repr:
                    removed += 1
                    continue
            keep.append(inst)
        blk.instructions[:] = keep
    return removed


def _prep_core(x, w, b, h):
    """Host-side shard prep for one core: pad/slab x, reshape w.

    Layout: partition p = cc*8 + yb. xa[p] = padded rows
    [h*32+yb*4, +10) of channels {g*16+cc}, shape [ROWS, G, XC].
    xb = xa shifted right by one column (for odd-kw 4B alignment).
    wr[p] = weight[b,0,cc,:,h*32+yb*4:+4,:], shape [KK, YQ, W].
    """
    xpad = np.zeros((C, H + 2 * PAD, XC), dtype=np.float32)
    xpad[:, PAD : PAD + H, PAD : PAD + W] = x[b]
    v = xpad[:, h * HHALF : h * HHALF + HHALF + 2 * PAD, :]  # [C, 38, 70]
    vr = v.reshape(G, WC, HHALF + 2 * PAD, XC)  # [g, cc, 38, 70]
    xa = np.empty((WC, NYB, ROWS, G, XC), dtype=np.float32)
    for yb in range(NYB):
        # [g, cc, 10, 70] -> [cc, 10, g, 70]
        xa[:, yb] = vr[:, :, yb * YQ : yb * YQ + ROWS, :].transpose(1, 2, 0, 3)
    xa = xa.reshape(PART, -1)
    xb = np.zeros_like(xa)
    xb3 = xb.reshape(PART, ROWS * G, XC)
    xb3[:, :, 1:] = xa.reshape(PART, ROWS * G, XC)[:, :, :-1]
    # weights: [16, 49, 32, 64] -> [cc, yb, pos, yq, x] -> [128, 49*4*64]
    # with the tap axis permuted into TAP_SEQ order
    wsh = w[b, 0, :, :, h * HHALF : (h + 1) * HHALF, :]
    wr = wsh.reshape(WC, KK, NYB, YQ, W).transpose(0, 2, 1, 3, 4)
    perm = [kh * K + kw for kh, kw in TAP_SEQ]
    wr = wr[:, :, perm, :, :]
    return (
        xa.astype(_BF16),
        xb.astype(_BF16),
        np.ascontiguousarray(wr).reshape(PART, -1).astype(_BF16),
    )


def _prep_inputs(x, weight):
    ident = np.eye(PART, dtype=_BF16)
    in_maps = []
    for core in range(NCORES):
        b, h = core // 2, core % 2
        xa, xb, wr = _prep_core(x, weight, b, h)
        in_maps.append({"xa": xa, "xb": xb, "wr": wr, "ident": ident})
    return in_maps


def _unpack_out(results):
    out = np.empty((B, C, H, W), dtype=np.float32)
    for core in range(NCORES):
        b, h = core // 2, core % 2
        # [cc, yb, yq, gh, g8, x] -> c = gh*128 + g8*16 + cc, y = yb*4+yq
        o = results[core]["out"].astype(np.float32).reshape(WC, NYB, YQ, 2, 8, W)
        o = o.transpose(3, 4, 0, 1, 2, 5).reshape(C, HHALF, W)
        out[b, :, h * HHALF : (h + 1) * HHALF, :] = o
    return out


def kernel(x: np.ndarray, weight: np.ndarray) -> np.ndarray:
    from concourse.bass_utils import run_bass_kernel_spmd

    if "nc" not in _cache:
        _cache["nc"] = _build()
    nc = _cache["nc"]

    in_maps = _prep_inputs(x, weight)
    res = run_bass_kernel_spmd(nc, in_maps, list(range(NCORES)))
    return _unpack_out(res.results)


# revision 32
# speedup vs baseline: 1.0131x; 1.0049x over previous
import sys

for _p in ("/root/.axon_site/_ro/trn_rl_repo", "/opt/trn_rl_repo"):
    if _p not in sys.path:
        sys.path.insert(0, _p)

import numpy as np
import ml_dtypes

# Problem constants (nn_LocalConvolution): x [4,256,64,64] f32,
# weight [4,1,16,49,64,64] f32, K=7, pad=3, stride=1, dil=1.
# out[b, g*16+cc, y, x] = sum_k x_pad[b, g*16+cc, y+kh-3, x+kw-3] * w[b,0,cc,k,y,x]
B, C, H, W = 4, 256, 64, 64
WC, KK, K, PAD = 16, 49, 7, 3
G = C // WC  # 16 channel groups sharing each weight channel
NCORES = 8
HHALF = H // 2  # 32 output rows per core (B=4 x 2 H-halves = 8 shards)
PART = 128
# Partition p = cc*8 + yb: weight channel cc in [0,16), y-block yb in [0,8).
# Each partition computes 4 output rows (yq) x 16 groups (g) x 64 cols.
NYB = 8  # y-blocks per core
YQ = 4  # rows per y-block
ROWS = YQ + K - 1  # 10 halo rows of padded input per partition
XC = W + 2 * PAD  # 70 padded cols
FREE = YQ * G * W  # 4096 product elems per partition per tap
BANK = 512  # fp32 elems per PSUM bank
NBANK = FREE // BANK  # 8 banks

# Tap processing order: even kw first (xb only needed for odd kw, so its
# DMA is off the critical path), kh-major so weight chunk kh=0 unblocks
# the first taps.
TAP_SEQ = [(kh, kw) for kh in range(K) for kw in (0, 2, 4, 6)] + [
    (kh, kw) for kh in range(K) for kw in (1, 3, 5)
]
# Positions in TAP_SEQ executed on GPSIMD instead of DVE. Measured: GPSIMD
# tensor_tensor contends for the shared SBUF port and slows concurrent DVE
# ops ~4x — keep this empty.
GP_POS = frozenset()
# xa DMA row chunks: first taps (kh=0) only need rows 0-3
XCHUNKS = ((0, 4), (4, 7), (7, 10))
# weight DMA chunks over TAP_SEQ positions (tap-order layout in DRAM);
# tiny first chunk so tap 0 starts as early as possible
WCHUNKS = ((0, 1), (1, 4), (4, 10), (10, 16), (16, 22), (22, 28), (28, 36), (36, 49))

_BF16 = ml_dtypes.bfloat16
_cache = {}


def _build():
    import concourse.bacc as bacc
    import concourse.mybir as mybir
    import concourse.tile as tile

    nc = bacc.Bacc(None, target_bir_lowering=False)
    bf = mybir.dt.bfloat16
    f32 = mybir.dt.float32

    xa_d = nc.dram_tensor("xa", (PART, ROWS * G * XC), bf, kind="ExternalInput")
    xb_d = nc.dram_tensor("xb", (PART, ROWS * G * XC), bf, kind="ExternalInput")
    wr_d = nc.dram_tensor("wr", (PART, KK * YQ * W), bf, kind="ExternalInput")
    id_d = nc.dram_tensor("ident", (PART, PART), bf, kind="ExternalInput")
    out_d = nc.dram_tensor("out", (PART, FREE), bf, kind="ExternalOutput")

    with tile.TileContext(nc) as tc:
        with (
            tc.tile_pool(name="xpool", bufs=1) as xpool,
            tc.tile_pool(name="cpool", bufs=1) as cpool,
            tc.tile_pool(name="wpool", bufs=1) as wpool,
            tc.tile_pool(name="tpool", bufs=6) as tpool,
            tc.tile_pool(name="gpool", bufs=2) as gpool,
            tc.tile_pool(name="opool", bufs=1) as opool,
            tc.tile_pool(name="psum", bufs=1, space="PSUM") as ppool,
        ):
            xa_t = xpool.tile([PART, ROWS, G, XC], bf, tag="xa")
            xb_t = xpool.tile([PART, ROWS, G, XC], bf, tag="xb")
            id_t = cpool.tile([PART, PART], bf, tag="id")
            warm = cpool.tile([PART, 1], f32, tag="warm")
            # critical path: ident then xa on the sync queue; weights on the
            # scalar queue in parallel; xb (odd-kw phase) last.
            # All input DMAs on one ring (sync): FIFO order = bandwidth
            # priority, and a single ring gets the full HBM rate. Order:
            # ident, xa rows 0-3, first weight taps, then interleaved.
            w_tiles = {}
            w_of_pos = {}
            for lo, hi in WCHUNKS:
                w_t = wpool.tile([PART, hi - lo, YQ, 1, W], bf, tag=f"w{lo}")
                w_tiles[lo] = w_t
                for pos in range(lo, hi):
                    w_of_pos[pos] = (w_t, pos - lo)

            def w_dma(ci):
                lo, hi = WCHUNKS[ci]
                nc.sync.dma_start(
                    w_tiles[lo][:], wr_d[:, lo * YQ * W : hi * YQ * W]
                )

            def xa_dma(ci):
                r0, r1 = XCHUNKS[ci]
                nc.sync.dma_start(
                    xa_t[:, r0:r1, :, :], xa_d[:, r0 * G * XC : r1 * G * XC]
                )

            nc.sync.dma_start(id_t[:], id_d[:])
            w_dma(0)
            xa_dma(0)
            w_dma(1)
            xa_dma(1)
            w_dma(2)
            xa_dma(2)
            w_dma(3)
            nc.sync.dma_start(xb_t[:], xb_d[:])
            for ci in range(4, len(WCHUNKS)):
                w_dma(ci)

            # preload the ACT copy table set during the head so the tail's
            # PSUM->SBUF copies don't pay ACT_TABLE_LOAD
            nc.scalar.copy(warm[:], id_t[:, 0:1])

            acc = [
                ppool.tile([PART, BANK], f32, name=f"ps{j}", tag=f"ps{j}")
                for j in range(NBANK)
            ]

            def tap_src(pos):
                kh, kw = TAP_SEQ[pos]
                if kw % 2 == 0:
                    src = xa_t[:, kh : kh + YQ, :, kw : kw + W]
                else:
                    src = xb_t[:, kh : kh + YQ, :, kw + 1 : kw + 1 + W]
                w_t, i = w_of_pos[pos]
                wap = w_t[:, i, :, :, :].broadcast_to((PART, YQ, G, W))
                return src, wap

            gp_pos = sorted(GP_POS)
            gp_tmp = {}

            def gp_issue(pos):
                src, wap = tap_src(pos)
                tmp = gpool.tile([PART, YQ, G, W], bf, tag="gtmp")
                nc.gpsimd.tensor_mul(tmp[:], src, wap)
                gp_tmp[pos] = tmp

            if gp_pos:
                gp_issue(gp_pos[0])

            first_src, first_wap = tap_src(0)
            first_tmp = tpool.tile([PART, YQ, G, W], bf, tag="tmp")
            nc.vector.tensor_mul(first_tmp[:], first_src, first_wap)
            for j in range(NBANK):
                yq, gh = j // 2, j % 2
                nc.tensor.matmul(
                    acc[j][:],
                    id_t[:],
                    first_tmp[:, yq, gh * 8 : (gh + 1) * 8, :],
                    start=True,
                    stop=False,
                )
            for pos in range(1, KK - 1):
                if pos in GP_POS:
                    # product was issued one gp-slot ago; prefetch the next
                    i = gp_pos.index(pos)
                    if i + 1 < len(gp_pos):
                        gp_issue(gp_pos[i + 1])
                    tmp = gp_tmp.pop(pos)
                else:
                    src, wap = tap_src(pos)
                    tmp = tpool.tile([PART, YQ, G, W], bf, tag="tmp")
                    nc.vector.tensor_mul(tmp[:], src, wap)
                for j in range(NBANK):
                    yq, gh = j // 2, j % 2
                    nc.tensor.matmul(
                        acc[j][:],
                        id_t[:],
                        tmp[:, yq, gh * 8 : (gh + 1) * 8, :],
                        start=False,
                        stop=False,
                    )
            # last tap in four yq-quarters: each PSUM bank pair gets its
            # stop-matmul (and PSUM copy / out DMA) as early as possible
            kh, kw = TAP_SEQ[KK - 1]
            w_t, wi = w_of_pos[KK - 1]
            for q in range(YQ):
                if kw % 2 == 0:
                    src = xa_t[:, kh + q : kh + q + 1, :, kw : kw + W]
                else:
                    src = xb_t[:, kh + q : kh + q + 1, :, kw + 1 : kw + 1 + W]
                wap = w_t[:, wi, q : q + 1, :, :].broadcast_to((PART, 1, G, W))
                tmp = tpool.tile([PART, 1, G, W], bf, tag="tmp4")
                nc.vector.tensor_mul(tmp[:], src, wap)
                for gh in range(2):
                    nc.tensor.matmul(
                        acc[q * 2 + gh][:],
                        id_t[:],
                        tmp[:, 0, gh * 8 : (gh + 1) * 8, :],
                        start=False,
                        stop=True,
                    )

            ostg = opool.tile([PART, NBANK, BANK], bf, tag="ostg")
            for j in range(NBANK):
                if j % 2 == 0:
                    nc.scalar.copy(ostg[:, j, :], acc[j][:])
                else:
                    nc.vector.tensor_copy(ostg[:, j, :], acc[j][:])
            for half in range(2):
                lo, hi = half * 4, half * 4 + 4
                nc.sync.dma_start(
                    out_d[:, lo * BANK : hi * BANK], ostg[:, lo:hi, :]
                )

    _dedupe_ldweights(nc)
    _hoist_input_dmas(nc)
    nc.compile()
    return nc


def _hoist_input_dmas(nc):
    """Move the leading wait-free input DMA issues from the body block to
    before the SP engine's all-engine-barrier wait in the preamble block, so
    the transfers overlap the ~7us engine-start preamble."""
    blocks = nc.main_func.blocks
    if len(blocks) < 2:
        return 0
    b0, b1 = blocks[0], blocks[1]
    hoist, keep = [], []
    frontier = True
    for ins in b1.instructions:
        if (
            frontier
            and len(hoist) < 3
            and type(ins).__name__ == "InstDMACopy"
            and getattr(ins.engine, "value", None) == "SP"
            and not (ins.sync_info and ins.sync_info.on_wait)
        ):
            # only the first-tap critical path (ident, w00, xa rows 0-3):
            # each hoisted issue delays the SP engine's barrier entry ~0.6us,
            # stalling all engines' body start
            hoist.append(ins)
            continue
        if type(ins).__name__ == "InstDMACopy" and getattr(
            ins.engine, "value", None
        ) == "SP":
            # stop at the first waiting DMA: later ones may reuse sem lanes
            frontier = False
        keep.append(ins)
    if not hoist:
        return 0
    # insertion point: first SP instruction in b0 that waits (barrier entry)
    idx = None
    for i, ins in enumerate(b0.instructions):
        if (
            getattr(ins.engine, "value", None) == "SP"
            and ins.sync_info
            and ins.sync_info.on_wait
        ):
            idx = i
            break
    if idx is None:
        idx = len(b0.instructions)
    b0.instructions[idx:idx] = hoist
    b1.instructions[:] = keep
    return len(hoist)


def _dedupe_ldweights(nc):
    """All PE matmuls share one identity stationary; drop every InstLdweights
    after the first so the PE array keeps the loaded weights. Only removes
    LdWeights that carry no semaphore activity and whose AP matches the
    first one exactly."""
    first_repr = None
    removed = 0
    for blk in nc.main_func.blocks:
        keep = []
        for inst in blk.instructions:
            if type(inst).__name__ == "InstLdweights":
                si = inst.sync_info
                clean = si is None or (not si.on_wait and not si.on_update)
                r = repr(inst.ins[0])
                if first_repr is None:
                    first_repr = r
                elif clean and r == first_repr:
                    removed += 1
                    continue
            keep.append(inst)
        blk.instructions[:] = keep
    return removed


def _prep_core(x, w, b, h):
    """Host-side shard prep for one core: pad/slab x, reshape w.

    Layout: partition p = cc*8 + yb. xa[p] = padded rows
    [h*32+yb*4, +10) of channels {g*16+cc}, shape [ROWS, G, XC].
    xb = xa shifted right by one column (for odd-kw 4B alignment).
    wr[p] = weight[b,0,cc,:,h*32+yb*4:+4,:], shape [KK, YQ, W].
    """
    xpad = np.zeros((C, H + 2 * PAD, XC), dtype=np.float32)
    xpad[:, PAD : PAD + H, PAD : PAD + W] = x[b]
    v = xpad[:, h * HHALF : h * HHALF + HHALF + 2 * PAD, :]  # [C, 38, 70]
    vr = v.reshape(G, WC, HHALF + 2 * PAD, XC)  # [g, cc, 38, 70]
    xa = np.empty((WC, NYB, ROWS, G, XC), dtype=np.float32)
    for yb in range(NYB):
        # [g, cc, 10, 70] -> [cc, 10, g, 70]
        xa[:, yb] = vr[:, :, yb * YQ : yb * YQ + ROWS, :].transpose(1, 2, 0, 3)
    xa = xa.reshape(PART, -1)
    xb = np.zeros_like(xa)
    xb3 = xb.reshape(PART, ROWS * G, XC)
    xb3[:, :, 1:] = xa.reshape(PART, ROWS * G, XC)[:, :, :-1]
    # weights: [16, 49, 32, 64] -> [cc, yb, pos, yq, x] -> [128, 49*4*64]
    # with the tap axis permuted into TAP_SEQ order
    wsh = w[b, 0, :, :, h * HHALF : (h + 1) * HHALF, :]
    wr = wsh.reshape(WC, KK, NYB, YQ, W).transpose(0, 2, 1, 3, 4)
    perm = [kh * K + kw for kh, kw in TAP_SEQ]
    wr = wr[:, :, perm, :, :]
    return (
        xa.astype(_BF16),
        xb.astype(_BF16),
        np.ascontiguousarray(wr).reshape(PART, -1).astype(_BF16),
    )


def _prep_inputs(x, weight):
    ident = np.eye(PART, dtype=_BF16)
    in_maps = []
    for core in range(NCORES):
        b, h = core // 2, core % 2
        xa, xb, wr = _prep_core(x, weight, b, h)
        in_maps.append({"xa": xa, "xb": xb, "wr": wr, "ident": ident})
    return in_maps


def _unpack_out(results):
    out = np.empty((B, C, H, W), dtype=np.float32)
    for core in range(NCORES):
        b, h = core // 2, core % 2
        # [cc, yb, yq, gh, g8, x] -> c = gh*128 + g8*16 + cc, y = yb*4+yq
        o = results[core]["out"].astype(np.float32).reshape(WC, NYB, YQ, 2, 8, W)
        o = o.transpose(3, 4, 0, 1, 2, 5).reshape(C, HHALF, W)
        out[b, :, h * HHALF : (h + 1) * HHALF, :] = o
    return out


def kernel(x: np.ndarray, weight: np.ndarray) -> np.ndarray:
    from concourse.bass_utils import run_bass_kernel_spmd

    if "nc" not in _cache:
        _cache["nc"] = _build()
    nc = _cache["nc"]

    in_maps = _prep_inputs(x, weight)
    res = run_bass_kernel_spmd(nc, in_maps, list(range(NCORES)))
    return _unpack_out(res.results)


# revision 37
# speedup vs baseline: 1.0164x; 1.0033x over previous
import sys

for _p in ("/root/.axon_site/_ro/trn_rl_repo", "/opt/trn_rl_repo"):
    if _p not in sys.path:
        sys.path.insert(0, _p)

import numpy as np
import ml_dtypes

# Problem constants (nn_LocalConvolution): x [4,256,64,64] f32,
# weight [4,1,16,49,64,64] f32, K=7, pad=3, stride=1, dil=1.
# out[b, g*16+cc, y, x] = sum_k x_pad[b, g*16+cc, y+kh-3, x+kw-3] * w[b,0,cc,k,y,x]
B, C, H, W = 4, 256, 64, 64
WC, KK, K, PAD = 16, 49, 7, 3
G = C // WC  # 16 channel groups sharing each weight channel
NCORES = 8
HHALF = H // 2  # 32 output rows per core (B=4 x 2 H-halves = 8 shards)
PART = 128
# Partition p = cc*8 + yb: weight channel cc in [0,16), y-block yb in [0,8).
# Each partition computes 4 output rows (yq) x 16 groups (g) x 64 cols.
NYB = 8  # y-blocks per core
YQ = 4  # rows per y-block
ROWS = YQ + K - 1  # 10 halo rows of padded input per partition
XC = W + 2 * PAD  # 70 padded cols
FREE = YQ * G * W  # 4096 product elems per partition per tap
BANK = 512  # fp32 elems per PSUM bank
NBANK = FREE // BANK  # 8 banks

# Tap processing order: even kw first (xb only needed for odd kw, so its
# DMA is off the critical path), kh-major so weight chunk kh=0 unblocks
# the first taps.
TAP_SEQ = [(kh, kw) for kh in range(K) for kw in (0, 2, 4, 6)] + [
    (kh, kw) for kh in range(K) for kw in (1, 3, 5)
]
# Positions in TAP_SEQ executed on GPSIMD instead of DVE. Measured: GPSIMD
# tensor_tensor contends for the shared SBUF port and slows concurrent DVE
# ops ~4x — keep this empty.
GP_POS = frozenset()
# xa DMA row chunks: tap 0's first yq-half only needs rows 0-1
XCHUNKS = ((0, 2), (2, 4), (4, 7), (7, 10))
# weight DMA chunks over TAP_SEQ positions (tap-order layout in DRAM);
# tiny first chunk so tap 0 starts as early as possible
WCHUNKS = ((0, 1), (1, 4), (4, 10), (10, 16), (16, 22), (22, 28), (28, 36), (36, 49))

_BF16 = ml_dtypes.bfloat16
_cache = {}


def _build():
    import concourse.bacc as bacc
    import concourse.mybir as mybir
    import concourse.tile as tile

    nc = bacc.Bacc(None, target_bir_lowering=False)
    bf = mybir.dt.bfloat16
    f32 = mybir.dt.float32

    xa_d = nc.dram_tensor("xa", (PART, ROWS * G * XC), bf, kind="ExternalInput")
    xb_d = nc.dram_tensor("xb", (PART, ROWS * G * XC), bf, kind="ExternalInput")
    wr_d = nc.dram_tensor("wr", (PART, KK * YQ * W), bf, kind="ExternalInput")
    id_d = nc.dram_tensor("ident", (PART, PART), bf, kind="ExternalInput")
    out_d = nc.dram_tensor("out", (PART, FREE), bf, kind="ExternalOutput")

    with tile.TileContext(nc) as tc:
        with (
            tc.tile_pool(name="xpool", bufs=1) as xpool,
            tc.tile_pool(name="cpool", bufs=1) as cpool,
            tc.tile_pool(name="wpool", bufs=1) as wpool,
            tc.tile_pool(name="tpool", bufs=6) as tpool,
            tc.tile_pool(name="gpool", bufs=2) as gpool,
            tc.tile_pool(name="opool", bufs=1) as opool,
            tc.tile_pool(name="psum", bufs=1, space="PSUM") as ppool,
        ):
            xa_t = xpool.tile([PART, ROWS, G, XC], bf, tag="xa")
            xb_t = xpool.tile([PART, ROWS, G, XC], bf, tag="xb")
            id_t = cpool.tile([PART, PART], bf, tag="id")
            warm = cpool.tile([PART, 1], f32, tag="warm")
            # critical path: ident then xa on the sync queue; weights on the
            # scalar queue in parallel; xb (odd-kw phase) last.
            # All input DMAs on one ring (sync): FIFO order = bandwidth
            # priority, and a single ring gets the full HBM rate. Order:
            # ident, xa rows 0-3, first weight taps, then interleaved.
            w_tiles = {}
            w_of_pos = {}
            for lo, hi in WCHUNKS:
                w_t = wpool.tile([PART, hi - lo, YQ, 1, W], bf, tag=f"w{lo}")
                w_tiles[lo] = w_t
                for pos in range(lo, hi):
                    w_of_pos[pos] = (w_t, pos - lo)

            def w_dma(ci):
                lo, hi = WCHUNKS[ci]
                nc.sync.dma_start(
                    w_tiles[lo][:], wr_d[:, lo * YQ * W : hi * YQ * W]
                )

            def xa_dma(ci):
                r0, r1 = XCHUNKS[ci]
                nc.sync.dma_start(
                    xa_t[:, r0:r1, :, :], xa_d[:, r0 * G * XC : r1 * G * XC]
                )

            nc.sync.dma_start(id_t[:], id_d[:])
            w_dma(0)
            xa_dma(0)
            xa_dma(1)
            w_dma(1)
            xa_dma(2)
            w_dma(2)
            xa_dma(3)
            w_dma(3)
            nc.sync.dma_start(xb_t[:], xb_d[:])
            for ci in range(4, len(WCHUNKS)):
                w_dma(ci)

            # preload the ACT copy table set during the head so the tail's
            # PSUM->SBUF copies don't pay ACT_TABLE_LOAD
            nc.scalar.copy(warm[:], id_t[:, 0:1])

            acc = [
                ppool.tile([PART, BANK], f32, name=f"ps{j}", tag=f"ps{j}")
                for j in range(NBANK)
            ]

            def tap_src(pos):
                kh, kw = TAP_SEQ[pos]
                if kw % 2 == 0:
                    src = xa_t[:, kh : kh + YQ, :, kw : kw + W]
                else:
                    src = xb_t[:, kh : kh + YQ, :, kw + 1 : kw + 1 + W]
                w_t, i = w_of_pos[pos]
                wap = w_t[:, i, :, :, :].broadcast_to((PART, YQ, G, W))
                return src, wap

            gp_pos = sorted(GP_POS)
            gp_tmp = {}

            def gp_issue(pos):
                src, wap = tap_src(pos)
                tmp = gpool.tile([PART, YQ, G, W], bf, tag="gtmp")
                nc.gpsimd.tensor_mul(tmp[:], src, wap)
                gp_tmp[pos] = tmp

            if gp_pos:
                gp_issue(gp_pos[0])

            # first tap in two yq-halves: half A needs only xa rows 0-1, so
            # compute starts before the rows 2-3 chunk lands
            kh0, kw0 = TAP_SEQ[0]
            w_t0, wi0 = w_of_pos[0]
            for half in range(2):
                q0 = half * 2
                src = xa_t[:, kh0 + q0 : kh0 + q0 + 2, :, kw0 : kw0 + W]
                wap = w_t0[:, wi0, q0 : q0 + 2, :, :].broadcast_to(
                    (PART, 2, G, W)
                )
                tmp = tpool.tile([PART, 2, G, W], bf, tag="tmp2")
                nc.vector.tensor_mul(tmp[:], src, wap)
                for jj in range(4):
                    j = half * 4 + jj
                    yq, gh = jj // 2, jj % 2
                    nc.tensor.matmul(
                        acc[j][:],
                        id_t[:],
                        tmp[:, yq, gh * 8 : (gh + 1) * 8, :],
                        start=True,
                        stop=False,
                    )
            for pos in range(1, KK - 1):
                if pos in GP_POS:
                    # product was issued one gp-slot ago; prefetch the next
                    i = gp_pos.index(pos)
                    if i + 1 < len(gp_pos):
                        gp_issue(gp_pos[i + 1])
                    tmp = gp_tmp.pop(pos)
                else:
                    src, wap = tap_src(pos)
                    tmp = tpool.tile([PART, YQ, G, W], bf, tag="tmp")
                    nc.vector.tensor_mul(tmp[:], src, wap)
                for j in range(NBANK):
                    yq, gh = j // 2, j % 2
                    nc.tensor.matmul(
                        acc[j][:],
                        id_t[:],
                        tmp[:, yq, gh * 8 : (gh + 1) * 8, :],
                        start=False,
                        stop=False,
                    )
            # last tap in four yq-quarters: each PSUM bank pair gets its
            # stop-matmul (and PSUM copy / out DMA) as early as possible
            kh, kw = TAP_SEQ[KK - 1]
            w_t, wi = w_of_pos[KK - 1]
            for q in range(YQ):
                if kw % 2 == 0:
                    src = xa_t[:, kh + q : kh + q + 1, :, kw : kw + W]
                else:
                    src = xb_t[:, kh + q : kh + q + 1, :, kw + 1 : kw + 1 + W]
                wap = w_t[:, wi, q : q + 1, :, :].broadcast_to((PART, 1, G, W))
                tmp = tpool.tile([PART, 1, G, W], bf, tag="tmp4")
                nc.vector.tensor_mul(tmp[:], src, wap)
                for gh in range(2):
                    nc.tensor.matmul(
                        acc[q * 2 + gh][:],
                        id_t[:],
                        tmp[:, 0, gh * 8 : (gh + 1) * 8, :],
                        start=False,
                        stop=True,
                    )

            ostg = opool.tile([PART, NBANK, BANK], bf, tag="ostg")
            for j in range(NBANK):
                if j % 2 == 0:
                    nc.scalar.copy(ostg[:, j, :], acc[j][:])
                else:
                    nc.vector.tensor_copy(ostg[:, j, :], acc[j][:])
            # small final piece so the last DMA's transfer+receipt is short
            for lo, hi in ((0, 4), (4, 7), (7, 8)):
                nc.sync.dma_start(
                    out_d[:, lo * BANK : hi * BANK], ostg[:, lo:hi, :]
                )

    _dedupe_ldweights(nc)
    _hoist_input_dmas(nc)
    nc.compile()
    return nc


def _hoist_input_dmas(nc):
    """Move the leading wait-free input DMA issues from the body block to
    before the SP engine's all-engine-barrier wait in the preamble block, so
    the transfers overlap the ~7us engine-start preamble."""
    blocks = nc.main_func.blocks
    if len(blocks) < 2:
        return 0
    b0, b1 = blocks[0], blocks[1]
    hoist, keep = [], []
    frontier = True
    for ins in b1.instructions:
        if (
            frontier
            and len(hoist) < 4
            and type(ins).__name__ == "InstDMACopy"
            and getattr(ins.engine, "value", None) == "SP"
            and not (ins.sync_info and ins.sync_info.on_wait)
        ):
            # only the first-tap critical path (ident, w00, xa rows 0-3):
            # each hoisted issue delays the SP engine's barrier entry ~0.6us,
            # stalling all engines' body start
            hoist.append(ins)
            continue
        if type(ins).__name__ == "InstDMACopy" and getattr(
            ins.engine, "value", None
        ) == "SP":
            # stop at the first waiting DMA: later ones may reuse sem lanes
            frontier = False
        keep.append(ins)
    if not hoist:
        return 0
    # insertion point: first SP instruction in b0 that waits (barrier entry)
    idx = None
    for i, ins in enumerate(b0.instructions):
        if (
            getattr(ins.engine, "value", None) == "SP"
            and ins.sync_info
            and ins.sync_info.on_wait
        ):
            idx = i
            break
    if idx is None:
        idx = len(b0.instructions)
    b0.instructions[idx:idx] = hoist
    b1.instructions[:] = keep
    return len(hoist)


def _dedupe_ldweights(nc):
    """All PE matmuls share one identity stationary; drop every InstLdweights
    after the first so the PE array keeps the loaded weights. Only removes
    LdWeights that carry no semaphore activity and whose AP matches the
    first one exactly."""
    first_repr = None
    removed = 0
    for blk in nc.main_func.blocks:
        keep = []
        for inst in blk.instructions:
            if type(inst).__name__ == "InstLdweights":
                si = inst.sync_info
                clean = si is None or (not si.on_wait and not si.on_update)
                r = repr(inst.ins[0])
                if first_repr is None:
                    first_repr = r
                elif clean and r == first_repr:
                    removed += 1
                    continue
            keep.append(inst)
        blk.instructions[:] = keep
    return removed


def _prep_core(x, w, b, h):
    """Host-side shard prep for one core: pad/slab x, reshape w.

    Layout: partition p = cc*8 + yb. xa[p] = padded rows
    [h*32+yb*4, +10) of channels {g*16+cc}, shape [ROWS, G, XC].
    xb = xa shifted right by one column (for odd-kw 4B alignment).
    wr[p] = weight[b,0,cc,:,h*32+yb*4:+4,:], shape [KK, YQ, W].
    """
    xpad = np.zeros((C, H + 2 * PAD, XC), dtype=np.float32)
    xpad[:, PAD : PAD + H, PAD : PAD + W] = x[b]
    v = xpad[:, h * HHALF : h * HHALF + HHALF + 2 * PAD, :]  # [C, 38, 70]
    vr = v.reshape(G, WC, HHALF + 2 * PAD, XC)  # [g, cc, 38, 70]
    xa = np.empty((WC, NYB, ROWS, G, XC), dtype=np.float32)
    for yb in range(NYB):
        # [g, cc, 10, 70] -> [cc, 10, g, 70]
        xa[:, yb] = vr[:, :, yb * YQ : yb * YQ + ROWS, :].transpose(1, 2, 0, 3)
    xa = xa.reshape(PART, -1)
    xb = np.zeros_like(xa)
    xb3 = xb.reshape(PART, ROWS * G, XC)
    xb3[:, :, 1:] = xa.reshape(PART, ROWS * G, XC)[:, :, :-1]
    # weights: [16, 49, 32, 64] -> [cc, yb, pos, yq, x] -> [128, 49*4*64]
    # with the tap axis permuted into TAP_SEQ order
    wsh = w[b, 0, :, :, h * HHALF : (h + 1) * HHALF, :]
    wr = wsh.reshape(WC, KK, NYB, YQ, W).transpose(0, 2, 1, 3, 4)
    perm = [kh * K + kw for kh, kw in TAP_SEQ]
    wr = wr[:, :, perm, :, :]
    return (
        xa.astype(_BF16),
        xb.astype(_BF16),
        np.ascontiguousarray(wr).reshape(PART, -1).astype(_BF16),
    )


def _prep_inputs(x, weight):
    ident = np.eye(PART, dtype=_BF16)
    in_maps = []
    for core in range(NCORES):
        b, h = core // 2, core % 2
        xa, xb, wr = _prep_core(x, weight, b, h)
        in_maps.append({"xa": xa, "xb": xb, "wr": wr, "ident": ident})
    return in_maps


def _unpack_out(results):
    out = np.empty((B, C, H, W), dtype=np.float32)
    for core in range(NCORES):
        b, h = core // 2, core % 2
        # [cc, yb, yq, gh, g8, x] -> c = gh*128 + g8*16 + cc, y = yb*4+yq
        o = results[core]["out"].astype(np.float32).reshape(WC, NYB, YQ, 2, 8, W)
        o = o.transpose(3, 4, 0, 1, 2, 5).reshape(C, HHALF, W)
        out[b, :, h * HHALF : (h + 1) * HHALF, :] = o
    return out


def kernel(x: np.ndarray, weight: np.ndarray) -> np.ndarray:
    from concourse.bass_utils import run_bass_kernel_spmd

    if "nc" not in _cache:
        _cache["nc"] = _build()
    nc = _cache["nc"]

    in_maps = _prep_inputs(x, weight)
    res = run_bass_kernel_spmd(nc, in_maps, list(range(NCORES)))
    return _unpack_out(res.results)
